# revision 1
# baseline (speedup 1.0000x reference)
"""GAT regressor (2-layer GATConv + Linear) on 8 Trainium2 NeuronCores.

Sharding: nodes partitioned across 8 cores (core k owns rows
[k*N/8, (k+1)*N/8)); edges bucketed by dst core/block. Per layer each core
gathers source-node feature rows from a replicated DRAM table via
dma_gather, computes edge attention on-chip, and aggregates per-dst via
one-hot (selection-matrix) matmuls on the tensor engine. Between layers an
AllGather collective rebuilds the full layer-2 feature table.
"""
import os
import numpy as np
import ml_dtypes

import concourse.bacc as bacc
import concourse.bass as bass
import concourse.mybir as mybir
import concourse.tile as tile
from concourse.bass_utils import run_bass_kernel_spmd
from concourse.masks import make_identity

P = 128
NCORES = 8
CH = 32768            # dma_gather int16 chunk size (table rows per chunk)
MAXG_CALL = 8         # max groups (of 128 edges) per dma_gather call (>1024 idxs/call crashes HW)
BF = mybir.dt.bfloat16
F32 = mybir.dt.float32
bf16 = ml_dtypes.bfloat16

_CACHE = {}
_DEBUG = False
LAST_EXEC_NS = None
_STAGE = 3  # 0=tables only, 1=+L1 edges, 2=+allgather, 3=full
_SUB = 2   # within edge layer: 0=gather only, 1=+attn, 2=full
_NO_COLLECTIVE = False  # replace allgather with local copy (timing sims)


# ----------------------------------------------------------------- schedule
def _schedule(src, dst, N, own):
    """Bucket edges by (dst core, dst block, src chunk); pad each cell to a
    multiple of 128 with a group count common across cores.

    Returns (meta, per_core) where meta is compile-time structure shared by
    all cores and per_core holds idx/dlane arrays.
    """
    nblk = (own + P - 1) // P
    nchunk = (N + CH - 1) // CH
    core = dst // own
    local = dst - core * own
    blk = local // P
    lane = local % P
    chunk = src // CH

    # per-core cell counts [NCORES, nblk, nchunk]
    cell = np.zeros((NCORES, nblk, nchunk), np.int64)
    np.add.at(cell, (core, blk, chunk), 1)
    gcnt = (np.ceil(cell / P)).astype(np.int64).max(axis=0)  # [nblk, nchunk]

    # group metadata in processing order: super-blocks of 2 blocks, chunk-major
    calls = []   # (chunk, idx_col_off, n_idx, [(block, first, last), ...])
    order = []   # (block, chunk) per group in processing order
    col_off = 0
    for b0 in range(0, nblk, 2):
        blks = [b for b in (b0, b0 + 1) if b < nblk]
        for c in range(nchunk):
            groups = []
            for b in blks:
                g_in_blk = int(gcnt[b, c])
                prior = int(gcnt[b, :c].sum())
                tot = int(gcnt[b, :].sum())
                for j in range(g_in_blk):
                    first = (prior + j) == 0
                    last = (prior + j) == tot - 1
                    groups.append((b, first, last))
                    order.append((b, c))
            # split into calls of <= MAXG_CALL groups
            k = 0
            while k < len(groups):
                part = groups[k:k + MAXG_CALL]
                n_idx = len(part) * P
                calls.append((c, col_off, n_idx, part))
                col_off += n_idx // 16
                k += len(part)
    g_tot = len(order)
    meta = dict(nblk=nblk, nchunk=nchunk, calls=calls, g_tot=g_tot,
                idx_cols=col_off, gcnt=gcnt)

    # per-core slot arrays
    per_core = []
    for k in range(NCORES):
        m = core == k
        s_k, b_k, l_k, c_k = src[m], blk[m], lane[m], chunk[m]
        o = np.lexsort((c_k, b_k))
        s_k, b_k, l_k, c_k = s_k[o], b_k[o], l_k[o], c_k[o]
        cnt = np.zeros((nblk, nchunk), np.int64)
        np.add.at(cnt, (b_k, c_k), 1)
        # slot arrays in processing order
        idx_flat = np.zeros(g_tot * P, np.int64)
        lane_flat = np.full(g_tot * P, -1.0, np.float32)
        # fill: edges of cell (b,c) occupy the first cnt[b,c] slots of that
        # cell's group span; order of cells in slots follows processing order
        cell_starts = {}
        pos = 0
        seen = set()
        for g, (b, c) in enumerate(order):
            if (b, c) not in seen:
                seen.add((b, c))
                cell_starts[(b, c)] = g * P
        # edges are sorted by (b, c); compute per-edge slot
        edge_cell_rank = np.zeros(len(s_k), np.int64)
        start = 0
        for b in range(nblk):
            for c in range(nchunk):
                n = int(cnt[b, c])
                if n == 0:
                    continue
                sl = cell_starts[(b, c)]
                edge_cell_rank[start:start + n] = sl + np.arange(n)
                start += n
        idx_flat[edge_cell_rank] = s_k - c_k * CH
        lane_flat[edge_cell_rank] = l_k
        # pad slots keep idx 0 (valid for any chunk) and lane -1 (no one-hot)
        # idx image for dma_gather: int16, [16, n/16] wrap, 8x replicated
        img_cols = np.zeros((16, meta["idx_cols"]), np.int16)
        gcur = 0
        for (c, off, n_idx, part) in calls:
            n_g = len(part)
            vals = idx_flat[gcur * P:(gcur + n_g) * P].astype(np.int16)
            img_cols[:, off:off + n_idx // 16] = vals.reshape(-1, 16).T
            gcur += n_g
        idx_img = np.tile(img_cols, (8, 1))
        dlane = lane_flat.reshape(g_tot, P).T.copy()  # [128, g_tot] f32
        per_core.append(dict(idx_img=idx_img, dlane=dlane))
    return meta, per_core


# ------------------------------------------------------------------- build
def _build(meta, N, own, din, HH, CC):
    """Build the SPMD Bass program (same for all cores)."""
    nblk, nchunk = meta["nblk"], meta["nchunk"]
    calls, g_tot = meta["calls"], meta["g_tot"]
    HC = HH * CC
    R1 = 384 if HC == 256 else ((HC + HH + 127) // 128) * 128  # table1 cols
    R2 = 128 if CC == 64 else ((CC + 1 + 127) // 128) * 128    # table2 cols
    nt1 = ((N + P - 1) // P) * P       # padded table1 rows
    ntile1 = nt1 // P
    npad = nblk * P                    # padded own rows
    kch = din // P                     # k-chunks for layer-1 matmul

    nc = bacc.Bacc("TRN2", target_bir_lowering=False, debug=False,
                   num_devices=NCORES)
    dt = lambda n, s, d, k="ExternalInput": nc.dram_tensor(n, s, d, kind=k).ap()
    xT = dt("xT", [din, nt1], BF)
    xoT = dt("xoT", [din, npad], BF)
    rhs1 = dt("rhs1", [din, HC + HH], BF)        # [W1 | W1@As1]
    w1ad = dt("w1ad", [din, HH], BF)             # W1@Ad1
    rhs2 = dt("rhs2", [HC, CC + 2], BF)          # [W2 | W2@As2 | W2@Ad2]
    cs2 = dt("cs2", [P, CC + 2], F32)            # colsum(rhs2) replicated
    b1r = dt("b1r", [P, HC], F32)
    b2r = dt("b2r", [P, CC], F32)
    lwr = dt("lwr", [P, CC], F32)                # lin_w replicated
    yconst = dt("yconst", [P, 1], F32)           # lin_b - sum(lin_w)
    idx_img = dt("idx_img", [P, meta["idx_cols"]], mybir.dt.int16)
    dlane = dt("dlane", [P, g_tot], F32)
    y_out = dt("y_out", [P, nblk], F32, "ExternalOutput")
    dbg = dt("dbg", [P, nblk * 264], F32, "ExternalOutput") if _DEBUG else None
    dbg2 = dt("dbg2", [P, 384 + 256], F32, "ExternalOutput") if _DEBUG else None
    dbg3 = dt("dbg3", [P, 12 * 260], F32, "ExternalOutput") if _DEBUG else None

    with tile.TileContext(nc) as tc:
        with tc.tile_pool(name="const", bufs=1) as cpool, \
             tc.tile_pool(name="sb", bufs=3) as sb, \
             tc.tile_pool(name="stage", bufs=3) as stp, \
             tc.tile_pool(name="gpool", bufs=2) as gp, \
             tc.tile_pool(name="epi", bufs=2) as ep, \
             tc.tile_pool(name="psA", bufs=3, space="PSUM") as psA, \
             tc.tile_pool(name="psB", bufs=3, space="PSUM") as psB, \
             tc.tile_pool(name="psC", bufs=2, space="PSUM") as psC, \
             tc.tile_pool(name="dram", bufs=1, space="DRAM") as dram:

            table1 = dram.tile([nt1, R1], BF)
            t2slice = dram.tile([npad, R2], BF)
            table2 = dram.tile([N, R2], BF, addr_space="Shared")

            # ---- constants
            ident = cpool.tile([P, P], BF)
            make_identity(nc, ident[:])
            iota_row = cpool.tile([P, P], BF)
            nc.gpsimd.iota(iota_row[:], pattern=[[1, P]], base=0,
                           channel_multiplier=0,
                           allow_small_or_imprecise_dtypes=True)
            rhs1_t = cpool.tile([P, kch, HC + HH], BF)
            nc.sync.dma_start(out=rhs1_t[:], in_=rhs1[:].rearrange("(k p) c -> p k c", p=P))
            w1ad_t = cpool.tile([P, kch, HH], BF)
            nc.sync.dma_start(out=w1ad_t[:], in_=w1ad[:].rearrange("(k p) c -> p k c", p=P))
            rhs2_t = cpool.tile([P, HC // P, CC + 2], BF)
            nc.sync.dma_start(out=rhs2_t[:], in_=rhs2[:].rearrange("(k p) c -> p k c", p=P))
            cs2_t = cpool.tile([P, CC + 2], F32)
            nc.sync.dma_start(out=cs2_t[:], in_=cs2[:])
            b1_t = cpool.tile([P, HC], F32)
            nc.sync.dma_start(out=b1_t[:], in_=b1r[:])
            b2_t = cpool.tile([P, CC], F32)
            nc.sync.dma_start(out=b2_t[:], in_=b2r[:])
            lw_t = cpool.tile([P, CC], F32)
            nc.sync.dma_start(out=lw_t[:], in_=lwr[:])
            yc_t = cpool.tile([P, 1], F32)
            nc.sync.dma_start(out=yc_t[:], in_=yconst[:])
            dlane_t = cpool.tile([P, g_tot], F32)
            nc.sync.dma_start(out=dlane_t[:], in_=dlane[:])
            ad1_sb = cpool.tile([P, nblk * HH], BF)
            ad2_sb = cpool.tile([P, nblk], BF)
            y_sb = cpool.tile([P, nblk], F32)

            # ---- phase 1: build table1 rows [h1 | a_s1 | pad]
            SUP = 8
            for t0 in range(0, ntile1, SUP):
                nt = min(SUP, ntile1 - t0)
                lhs = sb.tile([P, kch, SUP * P], BF, tag="xload")
                for k in range(kch):
                    nc.sync.dma_start(
                        out=lhs[:, k, :nt * P],
                        in_=xT[k * P:(k + 1) * P, t0 * P:(t0 + nt) * P])
                stg = stp.tile([P, SUP, R1], BF, tag="stg1")
                if R1 > HC + HH:
                    nc.vector.memset(stg[:, :, HC + HH:], 0.0)
                for ti in range(nt):
                    ps = psB.tile([P, HC + HH], F32, tag="pB")
                    for k in range(kch):
                        nc.tensor.matmul(
                            ps[:], lhs[:, k, ti * P:(ti + 1) * P],
                            rhs1_t[:, k, :],
                            start=(k == 0), stop=(k == kch - 1))
                    if ti % 2 == 0:
                        nc.vector.tensor_copy(out=stg[:, ti, :HC + HH], in_=ps[:])
                    else:
                        nc.scalar.copy(out=stg[:, ti, :HC + HH], in_=ps[:])
                nc.sync.dma_start(
                    out=table1[t0 * P:(t0 + nt) * P, :].rearrange(
                        "(t p) c -> p t c", p=P),
                    in_=stg[:, :nt, :])

            # ---- phase 1b: a_d1 for owned nodes (local order)
            for b in range(nblk):
                lhs = sb.tile([P, kch, P], BF, tag="xoload")
                for k in range(kch):
                    nc.sync.dma_start(
                        out=lhs[:, k, :],
                        in_=xoT[k * P:(k + 1) * P, b * P:(b + 1) * P])
                ps = psC.tile([P, HH], F32, tag="pad1", bufs=2)
                for k in range(kch):
                    nc.tensor.matmul(ps[:], lhs[:, k, :],
                                     w1ad_t[:, k, :],
                                     start=(k == 0), stop=(k == kch - 1))
                nc.vector.tensor_copy(out=ad1_sb[:, b * HH:(b + 1) * HH], in_=ps[:])

            # ---- edge phases
            def edge_layer(layer):
                R = R1 if layer == 1 else R2
                nhead = HH if layer == 1 else 1
                ncol = HC if layer == 1 else CC
                table = table1 if layer == 1 else table2
                trows = nt1 if layer == 1 else N
                gcur = 0
                blk_ps = {}
                dbg_cnt = [0]
                for (c, off, n_idx, part) in calls:
                    n_g = len(part)
                    idx_t = sb.tile([P, MAXG_CALL * P // 16], mybir.dt.int16,
                                    tag="idx")
                    nc.sync.dma_start(out=idx_t[:, :n_idx // 16],
                                      in_=idx_img[:, off:off + n_idx // 16])
                    gb = gp.tile([P, MAXG_CALL, R], BF, tag=f"gb{layer}")
                    base = c * CH
                    hi = min(base + CH, trows)
                    nc.gpsimd.dma_gather(
                        gb[:, :n_g, :], table[base:hi, :],
                        idx_t[:, :n_idx // 16], n_idx, n_idx, R)
                    if _SUB == 0:
                        gcur += n_g
                        continue
                    # attention logits for the whole call
                    adps = psC.tile([P, MAXG_CALL * nhead], F32, tag="pad1",
                                    bufs=2)
                    sts = []
                    for gl, (b, first, last) in enumerate(part):
                        g = gcur + gl
                        st = sb.tile([P, P], BF, tag="st", bufs=MAXG_CALL + 2)
                        nc.vector.tensor_scalar(
                            st[:], iota_row[:], dlane_t[:, g:g + 1], None,
                            mybir.AluOpType.is_equal)
                        st_ps = psB.tile([P, P], BF, tag="pB")
                        nc.tensor.transpose(st_ps[:], st[:], ident[:])
                        s_sb = sb.tile([P, P], BF, tag="ssb", bufs=3)
                        nc.vector.tensor_copy(out=s_sb[:], in_=st_ps[:])
                        sts.append(st)
                        adcol = (ad1_sb[:, b * HH:(b + 1) * HH] if layer == 1
                                 else ad2_sb[:, b:b + 1])
                        nc.tensor.matmul(adps[:, gl * nhead:(gl + 1) * nhead],
                                         s_sb[:], adcol,
                                         start=True, stop=False)
                        nc.tensor.matmul(adps[:, gl * nhead:(gl + 1) * nhead],
                                         ident[:], gb[:, gl, ncol:ncol + nhead],
                                         start=False, stop=True)
                    ls = ep.tile([P, MAXG_CALL * nhead], F32, tag="ls")
                    lt = ep.tile([P, MAXG_CALL * nhead], F32, tag="lt")
                    nc.vector.tensor_scalar_mul(lt[:, :n_g * nhead],
                                                adps[:, :n_g * nhead], 0.2)
                    nc.vector.tensor_tensor(
                        out=ls[:, :n_g * nhead], in0=adps[:, :n_g * nhead],
                        in1=lt[:, :n_g * nhead], op=mybir.AluOpType.max)
                    wsb = ep.tile([P, MAXG_CALL * nhead], F32, tag="wsb")
                    nc.scalar.activation(wsb[:, :n_g * nhead],
                                         ls[:, :n_g * nhead],
                                         mybir.ActivationFunctionType.Exp)
                    wbf = ep.tile([P, MAXG_CALL * nhead], BF, tag="wbf")
                    nc.vector.tensor_copy(out=wbf[:, :n_g * nhead],
                                          in_=wsb[:, :n_g * nhead])
                    if _SUB == 1:
                        gcur += n_g
                        continue
                    for gl, (b, first, last) in enumerate(part):
                        wh = stp.tile([P, HC + HH], BF, tag="wh")
                        for h in range(nhead):
                            nc.vector.tensor_scalar_mul(
                                wh[:, h * CC:(h + 1) * CC],
                                gb[:, gl, h * CC:(h + 1) * CC],
                                wsb[:, gl * nhead + h:gl * nhead + h + 1])
                        nc.vector.tensor_copy(
                            out=wh[:, ncol:ncol + nhead],
                            in_=wbf[:, gl * nhead:(gl + 1) * nhead])
                        if first:
                            pb = psA.tile([P, HC + HH], F32, tag="pblk")
                            blk_ps[b] = pb
                        pb = blk_ps[b]

                        if _DEBUG and layer == 1 and gcur == 0 and gl == 1:
                            d2a = ep.tile([P, 384], F32, tag="d2a")
                            nc.vector.tensor_copy(out=d2a[:], in_=gb[:, 1, :])
                            nc.sync.dma_start(out=dbg2[:, :384], in_=d2a[:])
                            d2b = ep.tile([P, 256], F32, tag="d2b")
                            nc.vector.tensor_copy(out=d2b[:], in_=wh[:, :256])
                            nc.sync.dma_start(out=dbg2[:, 384:640], in_=d2b[:])
                            d2c = ep.tile([P, P], F32, tag="d2c")
                            nc.vector.tensor_copy(out=d2c[:], in_=sts[1][:])
                            nc.sync.dma_start(out=dbg3[:, 10 * 260:10 * 260 + 128],
                                              in_=d2c[:])
                            d2d = ep.tile([P, 4], F32, tag="d2d")
                            nc.vector.tensor_copy(out=d2d[:],
                                                  in_=wsb[:, 4:8])
                            nc.sync.dma_start(out=dbg3[:, 10 * 260 + 128:10 * 260 + 132],
                                              in_=d2d[:])
                        nc.tensor.matmul(pb[:, :ncol + nhead], sts[gl][:],
                                         wh[:, :ncol + nhead],
                                         start=first, stop=last,
                                         skip_group_check=True)
                        if _DEBUG and layer == 1 and b == 0:
                            gof = sum(1 for gg in range(gl + gcur)
                                      if True) if False else None
                        if _DEBUG and layer == 1 and b == 0:
                            slot = dbg_cnt[0]
                            dbg_cnt[0] += 1
                            if slot < 12:
                                d3 = ep.tile([P, 260], F32, tag="d3")
                                nc.vector.tensor_copy(out=d3[:], in_=pb[:, :260])
                                nc.sync.dma_start(
                                    out=dbg3[:, slot * 260:(slot + 1) * 260],
                                    in_=d3[:])
                        if last:
                            epilogue(layer, b, pb)
                            del blk_ps[b]
                    gcur += n_g

            def epilogue(layer, b, pb):
                nhead = HH if layer == 1 else 1
                ncol = HC if layer == 1 else CC
                if _DEBUG and layer == 1:
                    dcp = ep.tile([P, 264], F32, tag="dcp")
                    nc.vector.tensor_copy(out=dcp[:, :260], in_=pb[:])
                    nc.sync.dma_start(out=dbg[:, b * 264:b * 264 + 260],
                                      in_=dcp[:, :260])
                den = ep.tile([P, nhead], F32, tag="den")
                nc.vector.tensor_scalar_max(den[:], pb[:, ncol:ncol + nhead], 1e-30)
                rc = ep.tile([P, nhead], F32, tag="rc")
                nc.vector.reciprocal(rc[:], den[:])
                z = ep.tile([P, ncol], F32, tag="z")
                for h in range(nhead):
                    nc.vector.tensor_scalar_mul(
                        z[:, h * (ncol // nhead):(h + 1) * (ncol // nhead)],
                        pb[:, h * (ncol // nhead):(h + 1) * (ncol // nhead)],
                        rc[:, h:h + 1])
                bias = b1_t if layer == 1 else b2_t
                nc.vector.tensor_add(z[:], z[:], bias[:])
                # elu+1: t = relu(z) + exp(min(z,0))
                m = ep.tile([P, ncol], F32, tag="m")
                nc.vector.tensor_scalar_min(m[:], z[:], 0.0)
                e = ep.tile([P, ncol], F32, tag="e")
                nc.scalar.activation(e[:], m[:], mybir.ActivationFunctionType.Exp)
                r = ep.tile([P, ncol], F32, tag="r")
                nc.scalar.activation(r[:], z[:], mybir.ActivationFunctionType.Relu)
                t = ep.tile([P, ncol], BF if layer == 1 else F32, tag="t")
                nc.vector.tensor_add(t[:], e[:], r[:])
                if layer == 1:
                    # h2 row = (t-1) @ rhs2 = t@rhs2 - colsum(rhs2)
                    h2ps = psB.tile([P, CC + 2], F32, tag="pB")
                    for k in range(HC // P):
                        tt_ps = psB.tile([P, P], BF, tag="pB")
                        nc.tensor.transpose(tt_ps[:], t[:, k * P:(k + 1) * P],
                                            ident[:])
                        tt_sb = sb.tile([P, P], BF, tag="ttsb")
                        nc.vector.tensor_copy(out=tt_sb[:], in_=tt_ps[:])
                        nc.tensor.matmul(h2ps[:], tt_sb[:],
                                         rhs2_t[:, k, :],
                                         start=(k == 0), stop=(k == HC // P - 1))
                    h2r = ep.tile([P, CC + 2], BF, tag="h2r")
                    nc.vector.tensor_sub(h2r[:], h2ps[:], cs2_t[:])
                    nc.vector.tensor_copy(out=ad2_sb[:, b:b + 1],
                                          in_=h2r[:, CC + 1:CC + 2])
                    row2 = stp.tile([P, R2], BF, tag="row2")
                    nc.vector.memset(row2[:, CC + 1:], 0.0)
                    nc.vector.tensor_copy(out=row2[:, :CC + 1], in_=h2r[:, :CC + 1])
                    nc.sync.dma_start(out=t2slice[b * P:(b + 1) * P, :],
                                      in_=row2[:])
                else:
                    # y = (t-1)@lin_w + lin_b = sum(t*lw) + (lin_b - sum(lin_w))
                    q = ep.tile([P, CC], F32, tag="q")
                    nc.vector.tensor_mul(q[:], t[:], lw_t[:])
                    acc = ep.tile([P, 1], F32, tag="acc")
                    nc.vector.tensor_reduce(acc[:], q[:],
                                            axis=mybir.AxisListType.X,
                                            op=mybir.AluOpType.add)
                    nc.vector.tensor_add(y_sb[:, b:b + 1], acc[:], yc_t[:])

            if _STAGE >= 1:
                edge_layer(1)
            if _STAGE >= 2:
                if NCORES == 1 or _NO_COLLECTIVE:
                    # single core: table2 is just our own slice
                    cptmp = gp.tile([P, (own // P + 1) * R2], BF, tag="cptmp")
                    for r0 in range(0, own, P * 8):
                        nr = min(P * 8, own - r0)
                        ct = gp.tile([P, 8, R2], BF, tag="cptmp2")
                        nc.sync.dma_start(
                            out=ct[:, :nr // P, :],
                            in_=t2slice[r0:r0 + nr, :].rearrange(
                                "(t p) c -> p t c", p=P))
                        nc.sync.dma_start(
                            out=table2[r0:r0 + nr, :].rearrange(
                                "(t p) c -> p t c", p=P),
                            in_=ct[:, :nr // P, :])
                else:
                    # allgather layer-2 table
                    nc.gpsimd.collective_compute(
                        "AllGather", mybir.AluOpType.bypass,
                        replica_groups=[list(range(NCORES))],
                        ins=[t2slice[:own, :]], outs=[table2[:]])
            if _STAGE >= 3:
                edge_layer(2)
            else:
                nc.vector.memset(y_sb[:], 0.0)
            nc.sync.dma_start(out=y_out[:], in_=y_sb[:])

    nc.compile()
    return nc


# ------------------------------------------------------------------ kernel
def kernel(**inputs):
    x = np.asarray(inputs["x"], np.float32)
    ei = np.asarray(inputs["edge_index"])
    W1 = np.asarray(inputs["W1"], np.float32)
    att_s1 = np.asarray(inputs["att_s1"], np.float32)
    att_d1 = np.asarray(inputs["att_d1"], np.float32)
    b1 = np.asarray(inputs["b1"], np.float32)
    W2 = np.asarray(inputs["W2"], np.float32)
    att_s2 = np.asarray(inputs["att_s2"], np.float32)
    att_d2 = np.asarray(inputs["att_d2"], np.float32)
    b2 = np.asarray(inputs["b2"], np.float32)
    lin_w = np.asarray(inputs["lin_w"], np.float32)
    lin_b = np.asarray(inputs["lin_b"], np.float32)

    N, din = x.shape
    HH, CC = att_s1.shape
    HC = HH * CC
    own = N // NCORES
    loops = np.arange(N, dtype=np.int64)
    src = np.concatenate([ei[0].astype(np.int64), loops])
    dst = np.concatenate([ei[1].astype(np.int64), loops])

    key = (N, din, HH, CC, int(src.sum()) & 0xFFFFFFFF)
    if key not in _CACHE:
        meta, per_core = _schedule(src, dst, N, own)
        nc = _build(meta, N, own, din, HH, CC)
        _CACHE[key] = (nc, meta, per_core)
    nc, meta, per_core = _CACHE[key]

    nblk = meta["nblk"]
    nt1 = ((N + P - 1) // P) * P
    npad = nblk * P

    # host-side weight prep
    As1 = np.zeros((HC, HH), np.float32)
    Ad1 = np.zeros((HC, HH), np.float32)
    for h in range(HH):
        As1[h * CC:(h + 1) * CC, h] = att_s1[h]
        Ad1[h * CC:(h + 1) * CC, h] = att_d1[h]
    rhs1 = np.concatenate([W1, W1 @ As1], axis=1).astype(bf16)
    w1ad = (W1 @ Ad1).astype(bf16)
    rhs2 = np.concatenate([W2, W2 @ att_s2.T, W2 @ att_d2.T], axis=1)
    cs2 = np.tile(rhs2.astype(bf16).astype(np.float32).sum(0)[None, :], (P, 1)).astype(np.float32)
    rhs2 = rhs2.astype(bf16)
    b1r = np.tile(b1[None, :], (P, 1)).astype(np.float32)
    b2r = np.tile(b2[None, :], (P, 1)).astype(np.float32)
    lwr = np.tile(lin_w[:, 0][None, :], (P, 1)).astype(np.float32)
    yconst = np.full((P, 1), lin_b[0] - lin_w.sum(), np.float32)

    xTp = np.zeros((din, nt1), bf16)
    xTp[:, :N] = x.T.astype(bf16)
    common = dict(xT=xTp, rhs1=rhs1, w1ad=w1ad, rhs2=rhs2, cs2=cs2,
                  b1r=b1r, b2r=b2r, lwr=lwr, yconst=yconst)
    in_maps = []
    for k in range(NCORES):
        xo = np.zeros((din, npad), bf16)
        xo[:, :own] = x[k * own:(k + 1) * own].T.astype(bf16)
        in_maps.append(dict(common, xoT=xo,
                            idx_img=per_core[k]["idx_img"],
                            dlane=per_core[k]["dlane"]))

    trace = bool(os.environ.get("KERNEL_TRACE"))
    try:
        res = run_bass_kernel_spmd(nc, in_maps, core_ids=list(range(NCORES)),
                                   trace=trace)
    except ModuleNotFoundError:
        res = run_bass_kernel_spmd(nc, in_maps, core_ids=list(range(NCORES)))
    global LAST_EXEC_NS
    LAST_EXEC_NS = res.exec_time_ns
    y = np.empty(N, np.float32)
    for k in range(NCORES):
        yk = res.results[k]["y_out"]          # [128, nblk]
        y[k * own:(k + 1) * own] = yk.T.reshape(-1)[:own]
    return y



# revision 3
# speedup vs baseline: 1474.8735x; 1474.8735x over previous
"""GAT regressor (2-layer GATConv + Linear) on 8 Trainium2 NeuronCores.

Sharding: nodes partitioned across 8 cores (core k owns rows
[k*N/8, (k+1)*N/8)); edges bucketed by dst core/block. Per layer each core
gathers source-node feature rows from a replicated DRAM table via
dma_gather, computes edge attention on-chip, and aggregates per-dst via
one-hot (selection-matrix) matmuls on the tensor engine. Between layers an
AllGather collective rebuilds the full layer-2 feature table.
"""
import os
import numpy as np
import ml_dtypes

import concourse.bacc as bacc
import concourse.bass as bass
import concourse.mybir as mybir
import concourse.tile as tile
from concourse.bass_utils import run_bass_kernel_spmd
from concourse.masks import make_identity

P = 128
NCORES = 8
CH = 32768            # dma_gather int16 chunk size (table rows per chunk)
MAXG_CALL = 8         # max groups (of 128 edges) per dma_gather call (>1024 idxs/call crashes HW)
BF = mybir.dt.bfloat16
F32 = mybir.dt.float32
bf16 = ml_dtypes.bfloat16

_CACHE = {}
_DEBUG = False
LAST_EXEC_NS = None
_STAGE = 3  # 0=tables only, 1=+L1 edges, 2=+allgather, 3=full
_SUB = 2   # within edge layer: 0=gather only, 1=+attn, 2=full
_NO_COLLECTIVE = False  # replace allgather with local copy (timing sims)


# ----------------------------------------------------------------- schedule
def _schedule(src, dst, N, own):
    """Bucket edges by (dst core, dst block, src chunk); pad each cell to a
    multiple of 128 with a group count common across cores.

    Returns (meta, per_core) where meta is compile-time structure shared by
    all cores and per_core holds idx/dlane arrays.
    """
    nblk = (own + P - 1) // P
    nchunk = (N + CH - 1) // CH
    core = dst // own
    local = dst - core * own
    blk = local // P
    lane = local % P
    chunk = src // CH

    # per-core cell counts [NCORES, nblk, nchunk]
    cell = np.zeros((NCORES, nblk, nchunk), np.int64)
    np.add.at(cell, (core, blk, chunk), 1)
    gcnt = (np.ceil(cell / P)).astype(np.int64).max(axis=0)  # [nblk, nchunk]

    # group metadata in processing order: super-blocks of 2 blocks, chunk-major
    calls = []   # (chunk, idx_col_off, n_idx, [(block, first, last), ...])
    order = []   # (block, chunk) per group in processing order
    col_off = 0
    for b0 in range(0, nblk, 2):
        blks = [b for b in (b0, b0 + 1) if b < nblk]
        for c in range(nchunk):
            groups = []
            for b in blks:
                g_in_blk = int(gcnt[b, c])
                prior = int(gcnt[b, :c].sum())
                tot = int(gcnt[b, :].sum())
                for j in range(g_in_blk):
                    first = (prior + j) == 0
                    last = (prior + j) == tot - 1
                    groups.append((b, first, last))
                    order.append((b, c))
            # split into calls of <= MAXG_CALL groups
            k = 0
            while k < len(groups):
                part = groups[k:k + MAXG_CALL]
                n_idx = len(part) * P
                calls.append((c, col_off, n_idx, part))
                col_off += n_idx // 16
                k += len(part)
    g_tot = len(order)
    meta = dict(nblk=nblk, nchunk=nchunk, calls=calls, g_tot=g_tot,
                idx_cols=col_off, gcnt=gcnt)

    # per-core slot arrays
    per_core = []
    for k in range(NCORES):
        m = core == k
        s_k, b_k, l_k, c_k = src[m], blk[m], lane[m], chunk[m]
        o = np.lexsort((c_k, b_k))
        s_k, b_k, l_k, c_k = s_k[o], b_k[o], l_k[o], c_k[o]
        cnt = np.zeros((nblk, nchunk), np.int64)
        np.add.at(cnt, (b_k, c_k), 1)
        # slot arrays in processing order
        idx_flat = np.zeros(g_tot * P, np.int64)
        lane_flat = np.full(g_tot * P, -1.0, np.float32)
        # fill: edges of cell (b,c) occupy the first cnt[b,c] slots of that
        # cell's group span; order of cells in slots follows processing order
        cell_starts = {}
        pos = 0
        seen = set()
        for g, (b, c) in enumerate(order):
            if (b, c) not in seen:
                seen.add((b, c))
                cell_starts[(b, c)] = g * P
        # edges are sorted by (b, c); compute per-edge slot
        edge_cell_rank = np.zeros(len(s_k), np.int64)
        start = 0
        for b in range(nblk):
            for c in range(nchunk):
                n = int(cnt[b, c])
                if n == 0:
                    continue
                sl = cell_starts[(b, c)]
                edge_cell_rank[start:start + n] = sl + np.arange(n)
                start += n
        idx_flat[edge_cell_rank] = s_k - c_k * CH
        lane_flat[edge_cell_rank] = l_k
        # pad slots keep idx 0 (valid for any chunk) and lane -1 (no one-hot)
        # idx image for dma_gather: int16, [16, n/16] wrap, 8x replicated
        img_cols = np.zeros((16, meta["idx_cols"]), np.int16)
        gcur = 0
        for (c, off, n_idx, part) in calls:
            n_g = len(part)
            vals = idx_flat[gcur * P:(gcur + n_g) * P].astype(np.int16)
            img_cols[:, off:off + n_idx // 16] = vals.reshape(-1, 16).T
            gcur += n_g
        idx_img = np.tile(img_cols, (8, 1))
        dlane = lane_flat.reshape(g_tot, P).T.copy()  # [128, g_tot] f32
        per_core.append(dict(idx_img=idx_img, dlane=dlane))
    return meta, per_core


# ------------------------------------------------------------------- build
def _build(meta, N, own, din, HH, CC):
    """Build the SPMD Bass program (same for all cores)."""
    nblk, nchunk = meta["nblk"], meta["nchunk"]
    calls, g_tot = meta["calls"], meta["g_tot"]
    HC = HH * CC
    R1 = 384 if HC == 256 else ((HC + HH + 127) // 128) * 128  # table1 cols
    R2 = 128 if CC == 64 else ((CC + 1 + 127) // 128) * 128    # table2 cols
    nt1 = ((N + P - 1) // P) * P       # padded table1 rows
    ntile1 = nt1 // P
    npad = nblk * P                    # padded own rows
    kch = din // P                     # k-chunks for layer-1 matmul

    nc = bacc.Bacc("TRN2", target_bir_lowering=False, debug=False,
                   num_devices=NCORES)
    dt = lambda n, s, d, k="ExternalInput": nc.dram_tensor(n, s, d, kind=k).ap()
    xT = dt("xT", [din, nt1], BF)
    xoT = dt("xoT", [din, npad], BF)
    rhs1 = dt("rhs1", [din, HC + HH], BF)        # [W1 | W1@As1]
    w1ad = dt("w1ad", [din, HH], BF)             # W1@Ad1
    rhs2 = dt("rhs2", [HC, CC + 2], BF)          # [W2 | W2@As2 | W2@Ad2]
    cs2 = dt("cs2", [P, CC + 2], F32)            # colsum(rhs2) replicated
    b1r = dt("b1r", [P, HC], F32)
    b2r = dt("b2r", [P, CC], F32)
    lwr = dt("lwr", [P, CC], F32)                # lin_w replicated
    yconst = dt("yconst", [P, 1], F32)           # lin_b - sum(lin_w)
    idx_img = dt("idx_img", [P, meta["idx_cols"]], mybir.dt.int16)
    dlane = dt("dlane", [P, g_tot], F32)
    y_out = dt("y_out", [P, nblk], F32, "ExternalOutput")
    dbg = dt("dbg", [P, nblk * 264], F32, "ExternalOutput") if _DEBUG else None
    dbg2 = dt("dbg2", [P, 384 + 256], F32, "ExternalOutput") if _DEBUG else None
    dbg3 = dt("dbg3", [P, 12 * 260], F32, "ExternalOutput") if _DEBUG else None

    with tile.TileContext(nc) as tc:
        with tc.tile_pool(name="const", bufs=1) as cpool, \
             tc.tile_pool(name="sb", bufs=3) as sb, \
             tc.tile_pool(name="stage", bufs=3) as stp, \
             tc.tile_pool(name="gpool", bufs=2) as gp, \
             tc.tile_pool(name="epi", bufs=2) as ep, \
             tc.tile_pool(name="psA", bufs=3, space="PSUM") as psA, \
             tc.tile_pool(name="psB", bufs=3, space="PSUM") as psB, \
             tc.tile_pool(name="psC", bufs=2, space="PSUM") as psC, \
             tc.tile_pool(name="dram", bufs=1, space="DRAM") as dram:

            table1 = dram.tile([nt1, R1], BF)
            t2slice = dram.tile([npad, R2], BF)
            table2 = dram.tile([N, R2], BF, addr_space="Shared")

            # ---- constants
            ident = cpool.tile([P, P], BF)
            make_identity(nc, ident[:])
            iota_row = cpool.tile([P, P], BF)
            nc.gpsimd.iota(iota_row[:], pattern=[[1, P]], base=0,
                           channel_multiplier=0,
                           allow_small_or_imprecise_dtypes=True)
            rhs1_t = cpool.tile([P, kch, HC + HH], BF)
            nc.sync.dma_start(out=rhs1_t[:], in_=rhs1[:].rearrange("(k p) c -> p k c", p=P))
            w1ad_t = cpool.tile([P, kch, HH], BF)
            nc.sync.dma_start(out=w1ad_t[:], in_=w1ad[:].rearrange("(k p) c -> p k c", p=P))
            rhs2_t = cpool.tile([P, HC // P, CC + 2], BF)
            nc.sync.dma_start(out=rhs2_t[:], in_=rhs2[:].rearrange("(k p) c -> p k c", p=P))
            cs2_t = cpool.tile([P, CC + 2], F32)
            nc.sync.dma_start(out=cs2_t[:], in_=cs2[:])
            b1_t = cpool.tile([P, HC], F32)
            nc.sync.dma_start(out=b1_t[:], in_=b1r[:])
            b2_t = cpool.tile([P, CC], F32)
            nc.sync.dma_start(out=b2_t[:], in_=b2r[:])
            lw_t = cpool.tile([P, CC], F32)
            nc.sync.dma_start(out=lw_t[:], in_=lwr[:])
            yc_t = cpool.tile([P, 1], F32)
            nc.sync.dma_start(out=yc_t[:], in_=yconst[:])
            dlane_t = cpool.tile([P, g_tot], F32)
            nc.sync.dma_start(out=dlane_t[:], in_=dlane[:])
            ad1_sb = cpool.tile([P, nblk * HH], BF)
            ad2_sb = cpool.tile([P, nblk], BF)
            y_sb = cpool.tile([P, nblk], F32)

            # ---- phase 1: build table1 rows [h1 | a_s1 | pad]
            SUP = 8
            for t0 in range(0, ntile1, SUP):
                nt = min(SUP, ntile1 - t0)
                lhs = sb.tile([P, kch, SUP * P], BF, tag="xload")
                for k in range(kch):
                    nc.sync.dma_start(
                        out=lhs[:, k, :nt * P],
                        in_=xT[k * P:(k + 1) * P, t0 * P:(t0 + nt) * P])
                stg = stp.tile([P, SUP, R1], BF, tag="stg1")
                if R1 > HC + HH:
                    nc.vector.memset(stg[:, :, HC + HH:], 0.0)
                for ti in range(nt):
                    ps = psB.tile([P, HC + HH], F32, tag="pB")
                    for k in range(kch):
                        nc.tensor.matmul(
                            ps[:], lhs[:, k, ti * P:(ti + 1) * P],
                            rhs1_t[:, k, :],
                            start=(k == 0), stop=(k == kch - 1))
                    if ti % 2 == 0:
                        nc.vector.tensor_copy(out=stg[:, ti, :HC + HH], in_=ps[:])
                    else:
                        nc.scalar.copy(out=stg[:, ti, :HC + HH], in_=ps[:])
                nc.sync.dma_start(
                    out=table1[t0 * P:(t0 + nt) * P, :].rearrange(
                        "(t p) c -> p t c", p=P),
                    in_=stg[:, :nt, :])

            # ---- phase 1b: a_d1 for owned nodes (local order)
            for b in range(nblk):
                lhs = sb.tile([P, kch, P], BF, tag="xoload")
                for k in range(kch):
                    nc.sync.dma_start(
                        out=lhs[:, k, :],
                        in_=xoT[k * P:(k + 1) * P, b * P:(b + 1) * P])
                ps = psC.tile([P, HH], F32, tag="pad1", bufs=2)
                for k in range(kch):
                    nc.tensor.matmul(ps[:], lhs[:, k, :],
                                     w1ad_t[:, k, :],
                                     start=(k == 0), stop=(k == kch - 1))
                nc.vector.tensor_copy(out=ad1_sb[:, b * HH:(b + 1) * HH], in_=ps[:])

            # ---- edge phases
            def edge_layer(layer):
                R = R1 if layer == 1 else R2
                nhead = HH if layer == 1 else 1
                ncol = HC if layer == 1 else CC
                table = table1 if layer == 1 else table2
                trows = nt1 if layer == 1 else N
                gcur = 0
                blk_ps = {}
                dbg_cnt = [0]
                for (c, off, n_idx, part) in calls:
                    n_g = len(part)
                    idx_t = sb.tile([P, MAXG_CALL * P // 16], mybir.dt.int16,
                                    tag="idx")
                    nc.sync.dma_start(out=idx_t[:, :n_idx // 16],
                                      in_=idx_img[:, off:off + n_idx // 16])
                    gb = gp.tile([P, MAXG_CALL, R], BF, tag=f"gb{layer}")
                    base = c * CH
                    hi = min(base + CH, trows)
                    nc.gpsimd.dma_gather(
                        gb[:, :n_g, :], table[base:hi, :],
                        idx_t[:, :n_idx // 16], n_idx, n_idx, R)
                    if _SUB == 0:
                        gcur += n_g
                        continue
                    # attention logits for the whole call
                    adps = psC.tile([P, MAXG_CALL * nhead], F32, tag="pad1",
                                    bufs=2)
                    sts = []
                    for gl, (b, first, last) in enumerate(part):
                        g = gcur + gl
                        st = sb.tile([P, P], BF, tag="st", bufs=MAXG_CALL + 2)
                        nc.vector.tensor_scalar(
                            st[:], iota_row[:], dlane_t[:, g:g + 1], None,
                            mybir.AluOpType.is_equal)
                        st_ps = psB.tile([P, P], BF, tag="pB")
                        nc.tensor.transpose(st_ps[:], st[:], ident[:])
                        s_sb = sb.tile([P, P], BF, tag="ssb", bufs=3)
                        nc.vector.tensor_copy(out=s_sb[:], in_=st_ps[:])
                        sts.append(st)
                        adcol = (ad1_sb[:, b * HH:(b + 1) * HH] if layer == 1
                                 else ad2_sb[:, b:b + 1])
                        nc.tensor.matmul(adps[:, gl * nhead:(gl + 1) * nhead],
                                         s_sb[:], adcol,
                                         start=True, stop=False)
                        nc.tensor.matmul(adps[:, gl * nhead:(gl + 1) * nhead],
                                         ident[:], gb[:, gl, ncol:ncol + nhead],
                                         start=False, stop=True)
                    ls = ep.tile([P, MAXG_CALL * nhead], F32, tag="ls")
                    lt = ep.tile([P, MAXG_CALL * nhead], F32, tag="lt")
                    nc.vector.tensor_scalar_mul(lt[:, :n_g * nhead],
                                                adps[:, :n_g * nhead], 0.2)
                    nc.vector.tensor_tensor(
                        out=ls[:, :n_g * nhead], in0=adps[:, :n_g * nhead],
                        in1=lt[:, :n_g * nhead], op=mybir.AluOpType.max)
                    wsb = ep.tile([P, MAXG_CALL * nhead], F32, tag="wsb")
                    nc.scalar.activation(wsb[:, :n_g * nhead],
                                         ls[:, :n_g * nhead],
                                         mybir.ActivationFunctionType.Exp)
                    wbf = ep.tile([P, MAXG_CALL * nhead], BF, tag="wbf")
                    nc.vector.tensor_copy(out=wbf[:, :n_g * nhead],
                                          in_=wsb[:, :n_g * nhead])
                    if _SUB == 1:
                        gcur += n_g
                        continue
                    for gl, (b, first, last) in enumerate(part):
                        wh = stp.tile([P, HC + HH], BF, tag="wh")
                        for h in range(nhead):
                            nc.vector.tensor_scalar_mul(
                                wh[:, h * CC:(h + 1) * CC],
                                gb[:, gl, h * CC:(h + 1) * CC],
                                wsb[:, gl * nhead + h:gl * nhead + h + 1])
                        nc.vector.tensor_copy(
                            out=wh[:, ncol:ncol + nhead],
                            in_=wbf[:, gl * nhead:(gl + 1) * nhead])
                        if first:
                            pb = psA.tile([P, HC + HH], F32, tag="pblk")
                            blk_ps[b] = pb
                        pb = blk_ps[b]

                        if _DEBUG and layer == 1 and gcur == 0 and gl == 1:
                            d2a = ep.tile([P, 384], F32, tag="d2a")
                            nc.vector.tensor_copy(out=d2a[:], in_=gb[:, 1, :])
                            nc.sync.dma_start(out=dbg2[:, :384], in_=d2a[:])
                            d2b = ep.tile([P, 256], F32, tag="d2b")
                            nc.vector.tensor_copy(out=d2b[:], in_=wh[:, :256])
                            nc.sync.dma_start(out=dbg2[:, 384:640], in_=d2b[:])
                            d2c = ep.tile([P, P], F32, tag="d2c")
                            nc.vector.tensor_copy(out=d2c[:], in_=sts[1][:])
                            nc.sync.dma_start(out=dbg3[:, 10 * 260:10 * 260 + 128],
                                              in_=d2c[:])
                            d2d = ep.tile([P, 4], F32, tag="d2d")
                            nc.vector.tensor_copy(out=d2d[:],
                                                  in_=wsb[:, 4:8])
                            nc.sync.dma_start(out=dbg3[:, 10 * 260 + 128:10 * 260 + 132],
                                              in_=d2d[:])
                        nc.tensor.matmul(pb[:, :ncol + nhead], sts[gl][:],
                                         wh[:, :ncol + nhead],
                                         start=first, stop=last,
                                         skip_group_check=True)
                        if _DEBUG and layer == 1 and b == 0:
                            gof = sum(1 for gg in range(gl + gcur)
                                      if True) if False else None
                        if _DEBUG and layer == 1 and b == 0:
                            slot = dbg_cnt[0]
                            dbg_cnt[0] += 1
                            if slot < 12:
                                d3 = ep.tile([P, 260], F32, tag="d3")
                                nc.vector.tensor_copy(out=d3[:], in_=pb[:, :260])
                                nc.sync.dma_start(
                                    out=dbg3[:, slot * 260:(slot + 1) * 260],
                                    in_=d3[:])
                        if last:
                            epilogue(layer, b, pb)
                            del blk_ps[b]
                    gcur += n_g

            def epilogue(layer, b, pb):
                nhead = HH if layer == 1 else 1
                ncol = HC if layer == 1 else CC
                if _DEBUG and layer == 1:
                    dcp = ep.tile([P, 264], F32, tag="dcp")
                    nc.vector.tensor_copy(out=dcp[:, :260], in_=pb[:])
                    nc.sync.dma_start(out=dbg[:, b * 264:b * 264 + 260],
                                      in_=dcp[:, :260])
                den = ep.tile([P, nhead], F32, tag="den")
                nc.vector.tensor_scalar_max(den[:], pb[:, ncol:ncol + nhead], 1e-30)
                rc = ep.tile([P, nhead], F32, tag="rc")
                nc.vector.reciprocal(rc[:], den[:])
                z = ep.tile([P, ncol], F32, tag="z")
                for h in range(nhead):
                    nc.vector.tensor_scalar_mul(
                        z[:, h * (ncol // nhead):(h + 1) * (ncol // nhead)],
                        pb[:, h * (ncol // nhead):(h + 1) * (ncol // nhead)],
                        rc[:, h:h + 1])
                bias = b1_t if layer == 1 else b2_t
                nc.vector.tensor_add(z[:], z[:], bias[:])
                # elu+1: t = relu(z) + exp(min(z,0))
                m = ep.tile([P, ncol], F32, tag="m")
                nc.vector.tensor_scalar_min(m[:], z[:], 0.0)
                e = ep.tile([P, ncol], F32, tag="e")
                nc.scalar.activation(e[:], m[:], mybir.ActivationFunctionType.Exp)
                r = ep.tile([P, ncol], F32, tag="r")
                nc.scalar.activation(r[:], z[:], mybir.ActivationFunctionType.Relu)
                t = ep.tile([P, ncol], BF if layer == 1 else F32, tag="t")
                nc.vector.tensor_add(t[:], e[:], r[:])
                if layer == 1:
                    # h2 row = (t-1) @ rhs2 = t@rhs2 - colsum(rhs2)
                    h2ps = psB.tile([P, CC + 2], F32, tag="pB")
                    for k in range(HC // P):
                        tt_ps = psB.tile([P, P], BF, tag="pB")
                        nc.tensor.transpose(tt_ps[:], t[:, k * P:(k + 1) * P],
                                            ident[:])
                        tt_sb = sb.tile([P, P], BF, tag="ttsb")
                        nc.vector.tensor_copy(out=tt_sb[:], in_=tt_ps[:])
                        nc.tensor.matmul(h2ps[:], tt_sb[:],
                                         rhs2_t[:, k, :],
                                         start=(k == 0), stop=(k == HC // P - 1))
                    h2r = ep.tile([P, CC + 2], BF, tag="h2r")
                    nc.vector.tensor_sub(h2r[:], h2ps[:], cs2_t[:])
                    nc.vector.tensor_copy(out=ad2_sb[:, b:b + 1],
                                          in_=h2r[:, CC + 1:CC + 2])
                    row2 = stp.tile([P, R2], BF, tag="row2")
                    nc.vector.memset(row2[:, CC + 1:], 0.0)
                    nc.vector.tensor_copy(out=row2[:, :CC + 1], in_=h2r[:, :CC + 1])
                    nc.sync.dma_start(out=t2slice[b * P:(b + 1) * P, :],
                                      in_=row2[:])
                else:
                    # y = (t-1)@lin_w + lin_b = sum(t*lw) + (lin_b - sum(lin_w))
                    q = ep.tile([P, CC], F32, tag="q")
                    nc.vector.tensor_mul(q[:], t[:], lw_t[:])
                    acc = ep.tile([P, 1], F32, tag="acc")
                    nc.vector.tensor_reduce(acc[:], q[:],
                                            axis=mybir.AxisListType.X,
                                            op=mybir.AluOpType.add)
                    nc.vector.tensor_add(y_sb[:, b:b + 1], acc[:], yc_t[:])

            if _STAGE >= 1:
                edge_layer(1)
            if _STAGE >= 2:
                if NCORES == 1 or _NO_COLLECTIVE:
                    # single core: table2 is just our own slice
                    cptmp = gp.tile([P, (own // P + 1) * R2], BF, tag="cptmp")
                    for r0 in range(0, own, P * 8):
                        nr = min(P * 8, own - r0)
                        ct = gp.tile([P, 8, R2], BF, tag="cptmp2")
                        nc.sync.dma_start(
                            out=ct[:, :nr // P, :],
                            in_=t2slice[r0:r0 + nr, :].rearrange(
                                "(t p) c -> p t c", p=P))
                        nc.sync.dma_start(
                            out=table2[r0:r0 + nr, :].rearrange(
                                "(t p) c -> p t c", p=P),
                            in_=ct[:, :nr // P, :])
                else:
                    # allgather layer-2 table
                    nc.gpsimd.collective_compute(
                        "AllGather", mybir.AluOpType.bypass,
                        replica_groups=[list(range(NCORES))],
                        ins=[t2slice[:own, :]], outs=[table2[:]])
            if _STAGE >= 3:
                edge_layer(2)
            else:
                nc.vector.memset(y_sb[:], 0.0)
            nc.sync.dma_start(out=y_out[:], in_=y_sb[:])

    nc.compile()
    return nc


# ------------------------------------------------------------------ kernel
def prepare(**inputs):
    x = np.asarray(inputs["x"], np.float32)
    ei = np.asarray(inputs["edge_index"])
    W1 = np.asarray(inputs["W1"], np.float32)
    att_s1 = np.asarray(inputs["att_s1"], np.float32)
    att_d1 = np.asarray(inputs["att_d1"], np.float32)
    b1 = np.asarray(inputs["b1"], np.float32)
    W2 = np.asarray(inputs["W2"], np.float32)
    att_s2 = np.asarray(inputs["att_s2"], np.float32)
    att_d2 = np.asarray(inputs["att_d2"], np.float32)
    b2 = np.asarray(inputs["b2"], np.float32)
    lin_w = np.asarray(inputs["lin_w"], np.float32)
    lin_b = np.asarray(inputs["lin_b"], np.float32)

    N, din = x.shape
    HH, CC = att_s1.shape
    HC = HH * CC
    own = N // NCORES
    loops = np.arange(N, dtype=np.int64)
    src = np.concatenate([ei[0].astype(np.int64), loops])
    dst = np.concatenate([ei[1].astype(np.int64), loops])

    key = (N, din, HH, CC, int(src.sum()) & 0xFFFFFFFF)
    if key not in _CACHE:
        meta, per_core = _schedule(src, dst, N, own)
        nc = _build(meta, N, own, din, HH, CC)
        _CACHE[key] = (nc, meta, per_core)
    nc, meta, per_core = _CACHE[key]

    nblk = meta["nblk"]
    nt1 = ((N + P - 1) // P) * P
    npad = nblk * P

    # host-side weight prep
    As1 = np.zeros((HC, HH), np.float32)
    Ad1 = np.zeros((HC, HH), np.float32)
    for h in range(HH):
        As1[h * CC:(h + 1) * CC, h] = att_s1[h]
        Ad1[h * CC:(h + 1) * CC, h] = att_d1[h]
    rhs1 = np.concatenate([W1, W1 @ As1], axis=1).astype(bf16)
    w1ad = (W1 @ Ad1).astype(bf16)
    rhs2 = np.concatenate([W2, W2 @ att_s2.T, W2 @ att_d2.T], axis=1)
    cs2 = np.tile(rhs2.astype(bf16).astype(np.float32).sum(0)[None, :], (P, 1)).astype(np.float32)
    rhs2 = rhs2.astype(bf16)
    b1r = np.tile(b1[None, :], (P, 1)).astype(np.float32)
    b2r = np.tile(b2[None, :], (P, 1)).astype(np.float32)
    lwr = np.tile(lin_w[:, 0][None, :], (P, 1)).astype(np.float32)
    yconst = np.full((P, 1), lin_b[0] - lin_w.sum(), np.float32)

    xTp = np.zeros((din, nt1), bf16)
    xTp[:, :N] = x.T.astype(bf16)
    common = dict(xT=xTp, rhs1=rhs1, w1ad=w1ad, rhs2=rhs2, cs2=cs2,
                  b1r=b1r, b2r=b2r, lwr=lwr, yconst=yconst)
    in_maps = []
    for k in range(NCORES):
        xo = np.zeros((din, npad), bf16)
        xo[:, :own] = x[k * own:(k + 1) * own].T.astype(bf16)
        in_maps.append(dict(common, xoT=xo,
                            idx_img=per_core[k]["idx_img"],
                            dlane=per_core[k]["dlane"]))
    return nc, in_maps


def kernel(**inputs):
    nc, in_maps = prepare(**inputs)
    N = np.asarray(inputs["x"]).shape[0]
    own = N // NCORES

    trace = bool(os.environ.get("KERNEL_TRACE"))
    try:
        res = run_bass_kernel_spmd(nc, in_maps, core_ids=list(range(NCORES)),
                                   trace=trace)
    except ModuleNotFoundError:
        res = run_bass_kernel_spmd(nc, in_maps, core_ids=list(range(NCORES)))
    global LAST_EXEC_NS
    LAST_EXEC_NS = res.exec_time_ns
    y = np.empty(N, np.float32)
    for k in range(NCORES):
        yk = res.results[k]["y_out"]          # [128, nblk]
        y[k * own:(k + 1) * own] = yk.T.reshape(-1)[:own]
    return y



# revision 53
# speedup vs baseline: 2387.6254x; 1.6189x over previous
"""GAT regressor (2-layer GATConv + Linear) on 8 Trainium2 NeuronCores.

Sharding: nodes partitioned across 8 cores (core k owns rows
[k*N/8, (k+1)*N/8)); edges bucketed by dst core/block. Per layer each core
gathers source-node feature rows from a replicated DRAM table via
dma_gather, computes edge attention on-chip, and aggregates per-dst via
one-hot (selection-matrix) matmuls on the tensor engine. Between layers an
AllGather collective rebuilds the full layer-2 feature table.
"""
import os
import numpy as np
import ml_dtypes

import concourse.bacc as bacc
import concourse.bass as bass
import concourse.mybir as mybir
import concourse.tile as tile
from concourse.bass_utils import run_bass_kernel_spmd
from concourse.masks import make_identity

P = 128
NCORES = 8
CH = 32768            # dma_gather int16 chunk size (table rows per chunk)
MAXG_CALL = 8         # max groups (of 128 edges) per dma_gather call (>1024 idxs/call crashes HW)
NSWQ = int(os.environ.get("KNSWQ", "4"))  # SWDGE queues; must divide the 8
                                          # DMASW sem lanes & use a global
                                          # gather counter so lanes align
DLW = 64              # groups per DMA-broadcast dlaneT window
BF = mybir.dt.bfloat16
F32 = mybir.dt.float32
bf16 = ml_dtypes.bfloat16

_CACHE = {}
_DEBUG = False
LAST_EXEC_NS = None
_STAGE = 3  # 0=tables only, 1=+L1 edges, 2=+allgather, 3=full
_SUB = 2   # within edge layer: 0=gather only, 1=+attn, 2=full
_NO_COLLECTIVE = False  # replace allgather with local copy (timing sims)


# ----------------------------------------------------------------- schedule
def _schedule(src, dst, N, own):
    """Bucket edges by (dst core, dst block, src chunk); pad each cell to a
    multiple of 128 with a group count common across cores.

    Returns (meta, per_core) where meta is compile-time structure shared by
    all cores and per_core holds idx/dlane arrays.
    """
    nblk = (own + P - 1) // P
    nchunk = (N + CH - 1) // CH
    core = dst // own
    local = dst - core * own
    blk = local // P
    lane = local % P
    chunk = src // CH

    # per-core cell counts [NCORES, nblk, nchunk]
    cell = np.zeros((NCORES, nblk, nchunk), np.int64)
    np.add.at(cell, (core, blk, chunk), 1)
    gcnt = (np.ceil(cell / P)).astype(np.int64).max(axis=0)  # [nblk, nchunk]

    # group metadata in processing order: super-blocks of 2 blocks, chunk-major
    calls = []   # (chunk, idx_col_off, n_idx, [(block, first, last), ...])
    order = []   # (block, chunk) per group in processing order
    col_off = 0
    for b0 in range(0, nblk, 2):
        blks = [b for b in (b0, b0 + 1) if b < nblk]
        for c in range(nchunk):
            groups = []
            for b in blks:
                g_in_blk = int(gcnt[b, c])
                prior = int(gcnt[b, :c].sum())
                tot = int(gcnt[b, :].sum())
                for j in range(g_in_blk):
                    first = (prior + j) == 0
                    last = (prior + j) == tot - 1
                    groups.append((b, first, last))
                    order.append((b, c))
            # split into calls of <= MAXG_CALL groups
            k = 0
            while k < len(groups):
                part = groups[k:k + MAXG_CALL]
                n_idx = len(part) * P
                calls.append((c, col_off, n_idx, part))
                col_off += n_idx // 16
                k += len(part)
    g_tot = len(order)
    meta = dict(nblk=nblk, nchunk=nchunk, calls=calls, g_tot=g_tot,
                idx_cols=col_off, gcnt=gcnt)

    # per-core slot arrays
    per_core = []
    for k in range(NCORES):
        m = core == k
        s_k, b_k, l_k, c_k = src[m], blk[m], lane[m], chunk[m]
        o = np.lexsort((c_k, b_k))
        s_k, b_k, l_k, c_k = s_k[o], b_k[o], l_k[o], c_k[o]
        cnt = np.zeros((nblk, nchunk), np.int64)
        np.add.at(cnt, (b_k, c_k), 1)
        # slot arrays in processing order
        idx_flat = np.zeros(g_tot * P, np.int64)
        lane_flat = np.full(g_tot * P, -1.0, np.float32)
        # fill: edges of cell (b,c) occupy the first cnt[b,c] slots of that
        # cell's group span; order of cells in slots follows processing order
        cell_starts = {}
        pos = 0
        seen = set()
        for g, (b, c) in enumerate(order):
            if (b, c) not in seen:
                seen.add((b, c))
                cell_starts[(b, c)] = g * P
        # edges are sorted by (b, c); compute per-edge slot
        edge_cell_rank = np.zeros(len(s_k), np.int64)
        start = 0
        for b in range(nblk):
            for c in range(nchunk):
                n = int(cnt[b, c])
                if n == 0:
                    continue
                sl = cell_starts[(b, c)]
                edge_cell_rank[start:start + n] = sl + np.arange(n)
                start += n
        idx_flat[edge_cell_rank] = s_k - c_k * CH
        lane_flat[edge_cell_rank] = l_k
        # pad slots keep idx 0 (valid for any chunk) and lane -1 (no one-hot)
        # idx image for dma_gather: int16, [16, n/16] wrap, 8x replicated
        img_cols = np.zeros((16, meta["idx_cols"]), np.int16)
        gcur = 0
        for (c, off, n_idx, part) in calls:
            n_g = len(part)
            vals = idx_flat[gcur * P:(gcur + n_g) * P].astype(np.int16)
            img_cols[:, off:off + n_idx // 16] = vals.reshape(-1, 16).T
            gcur += n_g
        idx_img = np.tile(img_cols, (8, 1))
        dlane = lane_flat.reshape(g_tot, P).T.copy()  # [128, g_tot] f32
        # flat per-group lane list for the DMA-broadcast stT build:
        # dlaneT[0, g*128+e] = dst lane of edge e of group g
        gw = (g_tot + DLW - 1) // DLW
        dlaneT = np.full((1, gw * DLW * P), -1.0, np.float32)
        dlaneT[0, :g_tot * P] = lane_flat
        dlaneT = dlaneT.astype(ml_dtypes.bfloat16)
        per_core.append(dict(idx_img=idx_img, dlane=dlane, dlaneT=dlaneT))
    return meta, per_core


# ------------------------------------------------------------------- build
def _build(meta, N, own, din, HH, CC):
    """Build the SPMD Bass program (same for all cores)."""
    nblk, nchunk = meta["nblk"], meta["nchunk"]
    calls, g_tot = meta["calls"], meta["g_tot"]
    HC = HH * CC
    R1 = 384 if HC == 256 else ((HC + HH + 127) // 128) * 128  # table1 cols
    R2 = 128 if CC == 64 else ((CC + 1 + 127) // 128) * 128    # table2 cols
    nt1 = ((N + P - 1) // P) * P       # padded table1 rows
    ntile1 = nt1 // P
    npad = nblk * P                    # padded own rows
    kch = din // P                     # k-chunks for layer-1 matmul

    nc = bacc.Bacc("TRN2", target_bir_lowering=False, debug=False,
                   num_devices=NCORES, num_swdge_queues=NSWQ)
    dt = lambda n, s, d, k="ExternalInput": nc.dram_tensor(n, s, d, kind=k).ap()
    xT = dt("xT", [din, nt1], BF)
    xoT = dt("xoT", [din, npad], BF)
    rhs1 = dt("rhs1", [din, HC + HH], BF)        # [W1 | W1@As1]
    w1ad = dt("w1ad", [din, HH], BF)             # W1@Ad1
    rhs2 = dt("rhs2", [HC, CC + 2], BF)          # [W2 | W2@As2 | W2@Ad2]
    cs2 = dt("cs2", [P, CC + 2], F32)            # colsum(rhs2) replicated
    b1r = dt("b1r", [P, HC], BF)
    b2r = dt("b2r", [P, CC], BF)
    lwr = dt("lwr", [P, CC], F32)                # lin_w replicated
    yconst = dt("yconst", [P, 1], F32)           # lin_b - sum(lin_w)
    idx_img = dt("idx_img", [P, meta["idx_cols"]], mybir.dt.int16)
    dlane = dt("dlane", [P, g_tot], F32)
    gw_tot = (g_tot + DLW - 1) // DLW
    dlaneT = dt("dlaneT", [1, gw_tot * DLW * P], BF)
    y_out = dt("y_out", [P, nblk], F32, "ExternalOutput")
    dbg = dt("dbg", [P, nblk * 264], F32, "ExternalOutput") if _DEBUG else None
    dbg2 = dt("dbg2", [P, 384 + 256], F32, "ExternalOutput") if _DEBUG else None
    dbg3 = dt("dbg3", [P, 12 * 260], F32, "ExternalOutput") if _DEBUG else None

    with tile.TileContext(nc) as tc:
        with tc.tile_pool(name="const", bufs=1) as cpool, \
             tc.tile_pool(name="sb", bufs=3) as sb, \
             tc.tile_pool(name="stage", bufs=3) as stp, \
             tc.tile_pool(name="gpool", bufs=2 * NSWQ) as gp, \
             tc.tile_pool(name="epi", bufs=2) as ep, \
             tc.tile_pool(name="psA", bufs=3, space="PSUM") as psA, \
             tc.tile_pool(name="psB", bufs=3, space="PSUM") as psB, \
             tc.tile_pool(name="psC", bufs=2, space="PSUM") as psC, \
             tc.tile_pool(name="dram", bufs=1, space="DRAM") as dram:

            table1 = dram.tile([nt1, R1], BF)
            t2slice = dram.tile([npad, R2], BF)
            if NCORES == 1 or _NO_COLLECTIVE:
                table2 = dram.tile([N, R2], BF)
            else:
                table2 = dram.tile([N, R2], BF, addr_space="Shared")

            # ---- constants
            ident = cpool.tile([P, P], BF)
            make_identity(nc, ident[:])
            iota_row = cpool.tile([P, P], BF)
            nc.gpsimd.iota(iota_row[:], pattern=[[1, P]], base=0,
                           channel_multiplier=0,
                           allow_small_or_imprecise_dtypes=True)
            rhs1_t = cpool.tile([P, kch, HC + HH], BF)
            nc.sync.dma_start(out=rhs1_t[:], in_=rhs1[:].rearrange("(k p) c -> p k c", p=P))
            w1ad_t = cpool.tile([P, kch, HH], BF)
            nc.sync.dma_start(out=w1ad_t[:], in_=w1ad[:].rearrange("(k p) c -> p k c", p=P))
            rhs2_t = cpool.tile([P, HC // P, CC + 2], BF)
            nc.sync.dma_start(out=rhs2_t[:], in_=rhs2[:].rearrange("(k p) c -> p k c", p=P))
            cs2_t = cpool.tile([P, CC + 2], F32)
            nc.sync.dma_start(out=cs2_t[:], in_=cs2[:])
            b1_t = cpool.tile([P, HC], BF)
            nc.sync.dma_start(out=b1_t[:], in_=b1r[:])
            b2_t = cpool.tile([P, CC], BF)
            nc.sync.dma_start(out=b2_t[:], in_=b2r[:])
            lw_t = cpool.tile([P, CC], F32)
            nc.sync.dma_start(out=lw_t[:], in_=lwr[:])
            yc_t = cpool.tile([P, 1], F32)
            nc.sync.dma_start(out=yc_t[:], in_=yconst[:])
            dlane_t = cpool.tile([P, g_tot], F32)
            nc.sync.dma_start(out=dlane_t[:], in_=dlane[:])
            iota_col = cpool.tile([P, 1], F32)
            nc.gpsimd.iota(iota_col[:], pattern=[[1, 1]], base=0,
                           channel_multiplier=1,
                           allow_small_or_imprecise_dtypes=True)
            idx_all = cpool.tile([P, meta["idx_cols"]], mybir.dt.int16)
            nc.sync.dma_start(out=idx_all[:], in_=idx_img[:])
            ad1_sb = cpool.tile([P, nblk * HH], BF)
            ad2_sb = cpool.tile([P, nblk], BF)
            y_sb = cpool.tile([P, nblk], F32)
            if _STAGE < 3:
                nc.vector.memset(y_sb[:], 0.0)

            # ---- phase 1: build table1 rows [h1 | a_s1 | pad]
            SUP = 8
            for t0 in range(0, ntile1, SUP):
                nt = min(SUP, ntile1 - t0)
                lhs = sb.tile([P, kch, SUP * P], BF, tag="xload")
                for k in range(kch):
                    nc.sync.dma_start(
                        out=lhs[:, k, :nt * P],
                        in_=xT[k * P:(k + 1) * P, t0 * P:(t0 + nt) * P])
                stg = stp.tile([P, SUP, R1], BF, tag="stg1")
                if R1 > HC + HH:
                    nc.vector.memset(stg[:, :, HC + HH:], 0.0)
                for ti in range(nt):
                    ps = psB.tile([P, HC + HH], F32, tag="pB")
                    for k in range(kch):
                        nc.tensor.matmul(
                            ps[:], lhs[:, k, ti * P:(ti + 1) * P],
                            rhs1_t[:, k, :],
                            start=(k == 0), stop=(k == kch - 1))
                    if ti % 2 == 0:
                        nc.vector.tensor_copy(out=stg[:, ti, :HC + HH], in_=ps[:])
                    else:
                        nc.scalar.copy(out=stg[:, ti, :HC + HH], in_=ps[:])
                nc.sync.dma_start(
                    out=table1[t0 * P:(t0 + nt) * P, :].rearrange(
                        "(t p) c -> p t c", p=P),
                    in_=stg[:, :nt, :])

            # ---- phase 1b: a_d1 for owned nodes (local order)
            BB = 8
            for b0 in range(0, nblk, BB):
                nb = min(BB, nblk - b0)
                lhs = sb.tile([P, kch, BB * P], BF, tag="xoload", bufs=2)
                for k in range(kch):
                    nc.sync.dma_start(
                        out=lhs[:, k, :nb * P],
                        in_=xoT[k * P:(k + 1) * P, b0 * P:(b0 + nb) * P])
                for bi in range(nb):
                    b = b0 + bi
                    ps = psC.tile([P, HH], F32, tag="pad1", bufs=2)
                    for k in range(kch):
                        nc.tensor.matmul(ps[:], lhs[:, k, bi * P:(bi + 1) * P],
                                         w1ad_t[:, k, :],
                                         start=(k == 0), stop=(k == kch - 1))
                    nc.vector.tensor_copy(out=ad1_sb[:, b * HH:(b + 1) * HH],
                                          in_=ps[:])

            # ---- edge phases
            gq = {"i": 0}  # global SWDGE gather counter (both layers): keeps
                           # queue_num aligned with tile's rotating DMASW sems

            def edge_layer(layer):
                R = R1 if layer == 1 else R2
                nhead = HH if layer == 1 else 1
                ncol = HC if layer == 1 else CC
                table = table1 if layer == 1 else table2
                trows = nt1 if layer == 1 else N
                gcur = 0
                blk_ps = {}
                dbg_cnt = [0]
                dlw = {"w": -1, "tile": None}

                def dl_bcast(g):
                    # [128,128] tile of group g's dst lanes, replicated down
                    # partitions via a DMA-broadcast window of DLW groups
                    w = g // DLW
                    if w != dlw["w"]:
                        tl = sb.tile([P, DLW * P], BF, tag="dlbc", bufs=2)
                        nc.sync.dma_start(
                            out=tl[:],
                            in_=dlaneT[0:1, w * DLW * P:(w + 1) * DLW * P]
                            .partition_broadcast(P).squeeze(1))
                        dlw["w"], dlw["tile"] = w, tl
                    o = (g % DLW) * P
                    return dlw["tile"][:, o:o + P]

                for call_i, (c, off, n_idx, part) in enumerate(calls):
                    n_g = len(part)
                    gb = gp.tile([P, MAXG_CALL, R], BF, tag=f"gb{layer}")
                    base = c * CH
                    hi = min(base + CH, trows)
                    nc.gpsimd.dma_gather(
                        gb[:, :n_g, :], table[base:hi, :],
                        idx_all[:, off:off + n_idx // 16], n_idx, n_idx, R,
                        queue_num=gq["i"] % NSWQ)
                    gq["i"] += 1
                    if _SUB == 0:
                        # keep gathers live under DCE: fold a column into y_sb
                        nc.vector.tensor_tensor(
                            out=y_sb[:, :1], in0=y_sb[:, :1],
                            in1=gb[:, 0, :1], op=mybir.AluOpType.add)
                        gcur += n_g
                        continue
                    # attention logits for the whole call
                    adps = psC.tile([P, MAXG_CALL * nhead], F32, tag="pad1",
                                    bufs=2)
                    # transposed one-hots for the whole call in ONE op: the
                    # bcast window holds all groups' dlane rows contiguously
                    # and the iota_col scalar is group-independent
                    g0 = gcur
                    stT = sb.tile([P, MAXG_CALL, P], BF, tag="sst", bufs=3)
                    done = 0
                    while done < n_g:
                        g = g0 + done
                        dl_bcast(g)  # ensure window g//DLW is loaded
                        take = min(n_g - done, (g // DLW + 1) * DLW - g)
                        o = (g % DLW) * P
                        nc.vector.tensor_scalar(
                            stT[:, done:done + take, :],
                            dlw["tile"][:, o:o + take * P].rearrange(
                                "p (g e) -> p g e", e=P),
                            iota_col[:], None, mybir.AluOpType.is_equal)
                        done += take
                    sts = []
                    for gl, (b, first, last) in enumerate(part):
                        g = gcur + gl
                        st = sb.tile([P, P], BF, tag="st", bufs=MAXG_CALL + 2)
                        nc.vector.tensor_scalar(
                            st[:], iota_row[:], dlane_t[:, g:g + 1], None,
                            mybir.AluOpType.is_equal)
                        sts.append(st)
                        adcol = (ad1_sb[:, b * HH:(b + 1) * HH] if layer == 1
                                 else ad2_sb[:, b:b + 1])
                        ascol = ncol if layer == 1 else CC + 1
                        nc.tensor.matmul(adps[:, gl * nhead:(gl + 1) * nhead],
                                         stT[:, gl, :], adcol,
                                         start=True, stop=False)
                        nc.tensor.matmul(adps[:, gl * nhead:(gl + 1) * nhead],
                                         ident[:], gb[:, gl, ascol:ascol + nhead],
                                         start=False, stop=True)
                    # w = exp(leaky_relu(logits, 0.2)): leaky on DVE, exp on Act
                    lt = ep.tile([P, MAXG_CALL * nhead], F32, tag="lt")
                    nc.vector.tensor_scalar_mul(lt[:, :n_g * nhead],
                                                adps[:, :n_g * nhead], 0.2)
                    ls = ep.tile([P, MAXG_CALL * nhead], F32, tag="ls")
                    nc.vector.tensor_tensor(
                        out=ls[:, :n_g * nhead], in0=adps[:, :n_g * nhead],
                        in1=lt[:, :n_g * nhead], op=mybir.AluOpType.max)
                    wsb = ep.tile([P, MAXG_CALL * nhead], F32, tag="wsb")
                    nc.scalar.activation(wsb[:, :n_g * nhead],
                                         ls[:, :n_g * nhead],
                                         mybir.ActivationFunctionType.Exp)
                    if layer == 1:
                        wsb_bf = ep.tile([P, MAXG_CALL * nhead], BF, tag="wsbb")
                        nc.vector.tensor_copy(out=wsb_bf[:, :n_g * nhead],
                                              in_=wsb[:, :n_g * nhead])
                    if _SUB == 1:
                        gcur += n_g
                        continue
                    for gl, (b, first, last) in enumerate(part):
                        if layer == 1:
                            # head-interleaved h cols: one fused scale op
                            wh = stp.tile([P, HC + HH], BF, tag="wh")
                            wg = wsb_bf[:, gl * nhead:(gl + 1) * nhead]
                            nc.vector.tensor_tensor(
                                out=wh[:, :HC].rearrange(
                                    "p (c h) -> p c h", h=HH),
                                in0=gb[:, gl, :HC].rearrange(
                                    "p (c h) -> p c h", h=HH),
                                in1=wg.unsqueeze(1).broadcast_to([P, CC, HH]),
                                op=mybir.AluOpType.mult)
                            nc.vector.tensor_copy(
                                out=wh[:, HC:HC + HH], in_=wg)
                        else:
                            # table2 row = [h2 | 1 | a_s2]: one scaled copy
                            # yields numerator cols + denominator col
                            wh = stp.tile([P, CC + 1], BF, tag="wh2")
                            nc.vector.tensor_scalar_mul(
                                wh[:, :CC + 1], gb[:, gl, :CC + 1],
                                wsb[:, gl:gl + 1])
                        if first:
                            pb = psA.tile([P, HC + HH], F32, tag="pblk")
                            blk_ps[b] = pb
                        pb = blk_ps[b]

                        if _DEBUG and layer == 1 and gcur == 0 and gl == 1:
                            d2a = ep.tile([P, 384], F32, tag="d2a")
                            nc.vector.tensor_copy(out=d2a[:], in_=gb[:, 1, :])
                            nc.sync.dma_start(out=dbg2[:, :384], in_=d2a[:])
                            d2b = ep.tile([P, 256], F32, tag="d2b")
                            nc.vector.tensor_copy(out=d2b[:], in_=wh[:, :256])
                            nc.sync.dma_start(out=dbg2[:, 384:640], in_=d2b[:])
                            d2c = ep.tile([P, P], F32, tag="d2c")
                            nc.vector.tensor_copy(out=d2c[:], in_=sts[1][:])
                            nc.sync.dma_start(out=dbg3[:, 10 * 260:10 * 260 + 128],
                                              in_=d2c[:])
                            d2d = ep.tile([P, 4], F32, tag="d2d")
                            nc.vector.tensor_copy(out=d2d[:],
                                                  in_=wsb[:, 4:8])
                            nc.sync.dma_start(out=dbg3[:, 10 * 260 + 128:10 * 260 + 132],
                                              in_=d2d[:])
                        nc.tensor.matmul(pb[:, :ncol + nhead], sts[gl][:],
                                         wh[:, :ncol + nhead],
                                         start=first, stop=last,
                                         skip_group_check=True)
                        if _DEBUG and layer == 1 and b == 0:
                            gof = sum(1 for gg in range(gl + gcur)
                                      if True) if False else None
                        if _DEBUG and layer == 1 and b == 0:
                            slot = dbg_cnt[0]
                            dbg_cnt[0] += 1
                            if slot < 12:
                                d3 = ep.tile([P, 260], F32, tag="d3")
                                nc.vector.tensor_copy(out=d3[:], in_=pb[:, :260])
                                nc.sync.dma_start(
                                    out=dbg3[:, slot * 260:(slot + 1) * 260],
                                    in_=d3[:])
                        if last:
                            epilogue(layer, b, pb)
                            del blk_ps[b]
                    gcur += n_g

            def epilogue(layer, b, pb):
                nhead = HH if layer == 1 else 1
                ncol = HC if layer == 1 else CC
                if _DEBUG and layer == 1:
                    dcp = ep.tile([P, 264], F32, tag="dcp")
                    nc.vector.tensor_copy(out=dcp[:, :260], in_=pb[:])
                    nc.sync.dma_start(out=dbg[:, b * 264:b * 264 + 260],
                                      in_=dcp[:, :260])
                den = ep.tile([P, nhead], F32, tag="den")
                nc.vector.tensor_scalar_max(den[:], pb[:, ncol:ncol + nhead], 1e-30)
                rc = ep.tile([P, nhead], F32, tag="rc")
                nc.vector.reciprocal(rc[:], den[:])
                z = ep.tile([P, ncol], F32, tag="z")
                if layer == 1:
                    # h cols are head-interleaved: per-head stride-HH views
                    pbv = pb[:, :ncol].rearrange("p (c h) -> p h c", h=nhead)
                    zv = z[:].rearrange("p (c h) -> p h c", h=nhead)
                    for h in range(nhead):
                        nc.scalar.activation(
                            zv[:, h, :], pbv[:, h, :],
                            mybir.ActivationFunctionType.Copy,
                            scale=rc[:, h:h + 1])
                else:
                    nc.scalar.activation(z[:], pb[:, :ncol],
                                         mybir.ActivationFunctionType.Copy,
                                         scale=rc[:, 0:1])
                bias = b1_t if layer == 1 else b2_t
                nc.vector.tensor_add(z[:], z[:], bias[:])
                # elu+1: t = relu(z) + exp(min(z,0))
                m = ep.tile([P, ncol], F32, tag="m")
                nc.vector.tensor_scalar_min(m[:], z[:], 0.0)
                e = ep.tile([P, ncol], F32, tag="e")
                nc.scalar.activation(e[:], m[:], mybir.ActivationFunctionType.Exp)
                r = ep.tile([P, ncol], F32, tag="r")
                nc.scalar.activation(r[:], z[:], mybir.ActivationFunctionType.Relu)
                t = ep.tile([P, ncol], BF if layer == 1 else F32, tag="t")
                nc.vector.tensor_add(t[:], e[:], r[:])
                if layer == 1:
                    # h2 row = (t-1) @ rhs2 = t@rhs2 - colsum(rhs2)
                    h2ps = psB.tile([P, CC + 2], F32, tag="pB")
                    for k in range(HC // P):
                        tt_ps = psB.tile([P, P], BF, tag="pB")
                        nc.tensor.transpose(tt_ps[:], t[:, k * P:(k + 1) * P],
                                            ident[:])
                        tt_sb = sb.tile([P, P], BF, tag="ttsb")
                        nc.vector.tensor_copy(out=tt_sb[:], in_=tt_ps[:])
                        nc.tensor.matmul(h2ps[:], tt_sb[:],
                                         rhs2_t[:, k, :],
                                         start=(k == 0), stop=(k == HC // P - 1))
                    h2r = ep.tile([P, CC + 2], BF, tag="h2r")
                    nc.vector.tensor_sub(h2r[:], h2ps[:], cs2_t[:])
                    nc.vector.tensor_copy(out=ad2_sb[:, b:b + 1],
                                          in_=h2r[:, CC + 1:CC + 2])
                    # row2 = [h2 | 1.0 | a_s2 | 0-pad]
                    row2 = stp.tile([P, R2], BF, tag="row2")
                    nc.vector.memset(row2[:, CC:], 0.0)
                    nc.vector.memset(row2[:, CC:CC + 1], 1.0)
                    nc.vector.tensor_copy(out=row2[:, :CC], in_=h2r[:, :CC])
                    nc.vector.tensor_copy(out=row2[:, CC + 1:CC + 2],
                                          in_=h2r[:, CC:CC + 1])
                    nc.sync.dma_start(out=t2slice[b * P:(b + 1) * P, :],
                                      in_=row2[:])
                else:
                    # y = (t-1)@lin_w + lin_b = sum(t*lw) + (lin_b - sum(lin_w))
                    q = ep.tile([P, CC], F32, tag="q")
                    nc.vector.tensor_mul(q[:], t[:], lw_t[:])
                    acc = ep.tile([P, 1], F32, tag="acc")
                    nc.vector.tensor_reduce(acc[:], q[:],
                                            axis=mybir.AxisListType.X,
                                            op=mybir.AluOpType.add)
                    nc.vector.tensor_add(y_sb[:, b:b + 1], acc[:], yc_t[:])

            if _STAGE >= 1:
                edge_layer(1)
            if _STAGE >= 2:
                if NCORES == 1 or _NO_COLLECTIVE:
                    # single core: table2 is just our own slice
                    for r0 in range(0, own, P):
                        nr = min(P, own - r0)
                        ct = gp.tile([P, R2], BF, tag="cptmp2")
                        nc.sync.dma_start(out=ct[:nr, :],
                                          in_=t2slice[r0:r0 + nr, :])
                        nc.sync.dma_start(out=table2[r0:r0 + nr, :],
                                          in_=ct[:nr, :])
                else:
                    # allgather layer-2 table
                    nc.gpsimd.collective_compute(
                        "AllGather", mybir.AluOpType.bypass,
                        replica_groups=[list(range(NCORES))],
                        ins=[t2slice[:own, :]], outs=[table2[:]])
            if _STAGE >= 3:
                edge_layer(2)
            if _STAGE == 0:
                # keep table1 build live under DCE
                tt = ep.tile([P, 1], BF, tag="tt0")
                nc.sync.dma_start(out=tt[:], in_=table1[0:P, 0:1])
                nc.vector.tensor_tensor(out=y_sb[:, :1], in0=y_sb[:, :1],
                                        in1=tt[:], op=mybir.AluOpType.add)
            nc.sync.dma_start(out=y_out[:], in_=y_sb[:])

    nc.compile()
    return nc


# ------------------------------------------------------------------ kernel
def prepare(**inputs):
    x = np.asarray(inputs["x"], np.float32)
    ei = np.asarray(inputs["edge_index"])
    W1 = np.asarray(inputs["W1"], np.float32)
    att_s1 = np.asarray(inputs["att_s1"], np.float32)
    att_d1 = np.asarray(inputs["att_d1"], np.float32)
    b1 = np.asarray(inputs["b1"], np.float32)
    W2 = np.asarray(inputs["W2"], np.float32)
    att_s2 = np.asarray(inputs["att_s2"], np.float32)
    att_d2 = np.asarray(inputs["att_d2"], np.float32)
    b2 = np.asarray(inputs["b2"], np.float32)
    lin_w = np.asarray(inputs["lin_w"], np.float32)
    lin_b = np.asarray(inputs["lin_b"], np.float32)

    N, din = x.shape
    HH, CC = att_s1.shape
    HC = HH * CC
    own = N // NCORES
    loops = np.arange(N, dtype=np.int64)
    src = np.concatenate([ei[0].astype(np.int64), loops])
    dst = np.concatenate([ei[1].astype(np.int64), loops])

    key = (N, din, HH, CC, int(src.sum()) & 0xFFFFFFFF)
    if key not in _CACHE:
        meta, per_core = _schedule(src, dst, N, own)
        nc = _build(meta, N, own, din, HH, CC)
        _CACHE[key] = (nc, meta, per_core)
    nc, meta, per_core = _CACHE[key]

    nblk = meta["nblk"]
    nt1 = ((N + P - 1) // P) * P
    npad = nblk * P

    # host-side weight prep
    As1 = np.zeros((HC, HH), np.float32)
    Ad1 = np.zeros((HC, HH), np.float32)
    for h in range(HH):
        As1[h * CC:(h + 1) * CC, h] = att_s1[h]
        Ad1[h * CC:(h + 1) * CC, h] = att_d1[h]
    # head-interleaved layout for layer-1 hidden cols: new col j = (head
    # j%HH, channel j//HH) -> old col (j%HH)*CC + j//HH
    perm = np.arange(HC)
    perm = (perm % HH) * CC + perm // HH
    rhs1 = np.concatenate([W1[:, perm], W1 @ As1], axis=1).astype(bf16)
    w1ad = (W1 @ Ad1).astype(bf16)
    rhs2 = np.concatenate([W2, W2 @ att_s2.T, W2 @ att_d2.T], axis=1)[perm, :]
    cs2 = np.tile(rhs2.astype(bf16).astype(np.float32).sum(0)[None, :], (P, 1)).astype(np.float32)
    rhs2 = rhs2.astype(bf16)
    b1r = np.tile(b1[perm][None, :], (P, 1)).astype(bf16)
    b2r = np.tile(b2[None, :], (P, 1)).astype(bf16)
    lwr = np.tile(lin_w[:, 0][None, :], (P, 1)).astype(np.float32)
    yconst = np.full((P, 1), lin_b[0] - lin_w.sum(), np.float32)

    xTp = np.zeros((din, nt1), bf16)
    xTp[:, :N] = x.T.astype(bf16)
    common = dict(xT=xTp, rhs1=rhs1, w1ad=w1ad, rhs2=rhs2, cs2=cs2,
                  b1r=b1r, b2r=b2r, lwr=lwr, yconst=yconst)
    in_maps = []
    for k in range(NCORES):
        xo = np.zeros((din, npad), bf16)
        xo[:, :own] = x[k * own:(k + 1) * own].T.astype(bf16)
        in_maps.append(dict(common, xoT=xo,
                            idx_img=per_core[k]["idx_img"],
                            dlane=per_core[k]["dlane"],
                            dlaneT=per_core[k]["dlaneT"]))
    return nc, in_maps


def kernel(**inputs):
    nc, in_maps = prepare(**inputs)
    N = np.asarray(inputs["x"]).shape[0]
    own = N // NCORES

    trace = bool(os.environ.get("KERNEL_TRACE"))
    try:
        res = run_bass_kernel_spmd(nc, in_maps, core_ids=list(range(NCORES)),
                                   trace=trace)
    except ModuleNotFoundError:
        res = run_bass_kernel_spmd(nc, in_maps, core_ids=list(range(NCORES)))
    global LAST_EXEC_NS
    LAST_EXEC_NS = res.exec_time_ns
    y = np.empty(N, np.float32)
    for k in range(NCORES):
        yk = res.results[k]["y_out"]          # [128, nblk]
        y[k * own:(k + 1) * own] = yk.T.reshape(-1)[:own]
    return y

